# revision 12
# baseline (speedup 1.0000x reference)
"""Trainium2 Bass kernel for the tied-weight Critic MLP.

Math (derived from the reference):
  x   = concat(inputs, actions)                  (B, 420), B = 8192
  s   = sum over 30 column-blocks of 14          (B, 14)
  y1  = s @ W1.T + b1                            (B, 512)
  h1  = relu(layernorm_512(y1))        [g1=1, beta1=0, LN over the 30x tile
                                        equals LN over one 512 block]
  y2  = h1 @ (30*W2).T + b2                      (B, 512)
  h2  = relu(layernorm_512(y2))
  V   = h2 @ (30*wV).T + bV                      (B, 1)
  out = tile(V, 30)                              (B, 30)

Sharding: pure data parallelism - batch 8192 split as 1024 rows on each of
8 NeuronCores; weights replicated.

Per-core layout (batch-major, 8 tiles of 128 rows, two groups of 4 tiles):
  All transposes ride the DMA engines in bf16 (PE transposes and fp32
  matmuls are 2-4x slower on the PE): the four s blocks of a group are
  packed 32 partitions apart with a ones column (bias fold) and flipped by
  a single DMA transpose; mm1 runs as 4 bf16 matmuls against a 4x
  replicated weight tile; h1 is emitted in bf16 by the fused
  scale/bias-ReLU activation and flipped by 4 DMA transposes per tile; mm2
  is a K=1 b2-broadcast matmul plus 4 accumulating bf16 matmuls; LayerNorm
  uses bn_stats/bn_aggr; V is a fp32 mul+reduce against broadcast wV.
  Emission is phase-ordered across each group so the engines pipeline.
"""

import numpy as np

N_CORES = 8
B_FULL = 8192
B_CORE = B_FULL // N_CORES  # 1024
P = 128
N_TILES = B_CORE // P  # 8
GROUP = 4  # tiles per phase group
N_AGENTS = 30
IN_F = 14
HID = 512
EPS = 1e-5

_cache = {}


def _build(bV: float, loop_n: int = 1):
    import concourse.bass as bass
    import concourse.tile as tile
    from concourse import bacc, mybir
    from concourse.bass import ts

    f32 = mybir.dt.float32
    bf16 = mybir.dt.bfloat16
    AF = mybir.ActivationFunctionType
    ALU = mybir.AluOpType

    nc = bacc.Bacc("TRN2")

    xin_d = nc.dram_tensor("xin", (B_CORE, 360), f32, kind="ExternalInput")
    xact_d = nc.dram_tensor("xact", (B_CORE, 60), f32, kind="ExternalInput")
    w1t_d = nc.dram_tensor("w1t", (IN_F + 1, HID), bf16, kind="ExternalInput")
    w2t_d = nc.dram_tensor("w2t", (HID, HID), bf16, kind="ExternalInput")
    b2r_d = nc.dram_tensor("b2r", (1, HID), bf16, kind="ExternalInput")
    wvr_d = nc.dram_tensor("wvr", (1, HID), f32, kind="ExternalInput")
    out_d = nc.dram_tensor("out", (B_CORE, N_AGENTS), f32, kind="ExternalOutput")

    def bcast(ap, p=P):
        return bass.AP(tensor=ap.tensor, offset=ap.offset, ap=[[0, p]] + ap.ap[1:])

    with tile.TileContext(nc) as tc:
        with (
            tc.tile_pool(name="singles", bufs=1) as singles,
            tc.tile_pool(name="xp", bufs=2 * GROUP) as xp,
            tc.tile_pool(name="sp", bufs=2) as sp,
            tc.tile_pool(name="hp", bufs=GROUP) as hp,
            tc.tile_pool(name="stat", bufs=2 * GROUP) as stat,
            tc.tile_pool(name="op", bufs=GROUP) as op,
            tc.tile_pool(name="ps_y", bufs=GROUP, space="PSUM") as ps_y,
        ):
            # ---- constants / replicated weights ----
            ones30 = singles.tile([P, N_AGENTS], f32)
            nc.vector.memset(ones30, 1.0)
            ones1 = singles.tile([1, P], bf16)
            nc.vector.memset(ones1, 1.0)
            eps_t = singles.tile([P, 1], f32)
            nc.vector.memset(eps_t, EPS)

            # w1t replicated at partitions 0/32/64/96 (matmul needs lhsT and
            # rhs on the same base partition; the 4 sT slices sit 32 apart)
            w1t = singles.tile([96 + IN_F + 1, HID], bf16)
            for a in range(GROUP):
                nc.sync.dma_start(
                    out=w1t[32 * a : 32 * a + IN_F + 1, :], in_=w1t_d[:, :]
                )
            w2sb = singles.tile([P, 4, HID], bf16)
            nc.sync.dma_start(
                out=w2sb, in_=w2t_d[:, :].rearrange("(c p) n -> p c n", p=P)
            )
            b2r = singles.tile([1, HID], bf16)
            nc.sync.dma_start(out=b2r, in_=b2r_d[:, :])
            wv_bc = singles.tile([P, HID], f32)
            nc.gpsimd.dma_start(out=wv_bc, in_=bcast(wvr_d[:, :]))

            def layer_norm_relu(y_in, h_out):
                st6 = stat.tile([P, 6], f32, tag="st6")
                nc.vector.bn_stats(st6, y_in)
                mv = stat.tile([P, 2], f32, tag="mv")
                nc.vector.bn_aggr(mv, st6)
                rstd = stat.tile([P, 1], f32, tag="rstd")
                nc.scalar.activation(rstd, mv[:, 1:2], AF.Sqrt, bias=eps_t, scale=1.0)
                nc.vector.reciprocal(rstd, rstd)
                # nm = (mean * rstd) * -1
                nm = stat.tile([P, 1], f32, tag="nm")
                nc.vector.tensor_scalar(
                    out=nm,
                    in0=mv[:, 0:1],
                    scalar1=rstd,
                    scalar2=-1.0,
                    op0=ALU.mult,
                    op1=ALU.mult,
                )
                # h = relu(y * rstd + nm) = relu((y - mean) * rstd)
                nc.scalar.activation(h_out, y_in, AF.Relu, bias=nm, scale=rstd)

            def group_body(g):
                t0 = g * GROUP
                # ---- phase A: load x, form s, pack + single DMA transpose ----
                x_ts = []
                for a in range(GROUP):
                    rows = slice((t0 + a) * P, (t0 + a + 1) * P)
                    x_t = xp.tile([P, 420], f32, tag="x")
                    nc.sync.dma_start(out=x_t[:, 0:360], in_=xin_d[rows, :])
                    nc.sync.dma_start(out=x_t[:, 360:420], in_=xact_d[rows, :])
                    x_ts.append(x_t)
                s_f = sp.tile([P, GROUP, IN_F], f32, tag="sf")
                for a in range(GROUP):
                    nc.vector.reduce_sum(
                        s_f[:, a, :],
                        x_ts[a][:, :].rearrange("p (a f) -> p f a", f=IN_F),
                        axis=mybir.AxisListType.X,
                    )
                # s4b columns: tile a at 32a..32a+13, ones column at 32a+14
                # (becomes the bias row of each lhsT slice after transpose)
                s4b = sp.tile([P, P], bf16, tag="s4b")
                nc.vector.memset(s4b, 1.0)
                s4b_v = s4b[:, :].rearrange("p (a q) -> p a q", q=32)
                nc.scalar.copy(out=s4b_v[:, :, 0:IN_F], in_=s_f)
                st4 = sp.tile([P, P], bf16, tag="st4")
                nc.sync.dma_start_transpose(st4, s4b)

                # ---- phase B: mm1 ----
                y1s = []
                for a in range(GROUP):
                    y1 = ps_y.tile([P, HID], f32, tag="y1")
                    nc.tensor.matmul(
                        y1,
                        st4[32 * a : 32 * a + IN_F + 1, :],
                        w1t[32 * a : 32 * a + IN_F + 1, :],
                        start=True,
                        stop=True,
                        tile_position=(32 * a, 0),
                    )
                    y1s.append(y1)

                # ---- phase C: LN1 + ReLU -> bf16 ----
                h1bs = []
                for a in range(GROUP):
                    h1b = hp.tile([P, HID], bf16, tag="h1b")
                    layer_norm_relu(y1s[a], h1b)
                    h1bs.append(h1b)

                # ---- phase D: h1.T via DMA transposes ----
                h1ts = []
                for a in range(GROUP):
                    h1t = hp.tile([P, 4, P], bf16, tag="h1t")
                    for j in range(4):
                        nc.sync.dma_start_transpose(h1t[:, j, :], h1bs[a][:, ts(j, P)])
                    h1ts.append(h1t)

                # ---- phase E: mm2 = b2 (K=1) + 4 accumulating matmuls ----
                y2s = []
                for a in range(GROUP):
                    y2 = ps_y.tile([P, HID], f32, tag="y2")
                    nc.tensor.matmul(y2, ones1, b2r, start=True, stop=False)
                    for j in range(4):
                        nc.tensor.matmul(
                            y2,
                            h1ts[a][:, j, :],
                            w2sb[:, j, :],
                            start=False,
                            stop=(j == 3),
                        )
                    y2s.append(y2)

                # ---- phase F: LN2 + ReLU ----
                h2s = []
                for a in range(GROUP):
                    h2 = hp.tile([P, HID], f32, tag="h2")
                    layer_norm_relu(y2s[a], h2)
                    h2s.append(h2)

                # ---- phase G: V, broadcast to 30 cols, store ----
                for a in range(GROUP):
                    rows = slice((t0 + a) * P, (t0 + a + 1) * P)
                    tmp = hp.tile([P, HID], f32, tag="tmp")
                    v_t = stat.tile([P, 1], f32, tag="v")
                    nc.vector.tensor_mul(tmp, h2s[a], wv_bc)
                    nc.vector.reduce_sum(v_t, tmp, axis=mybir.AxisListType.X)
                    o30 = op.tile([P, N_AGENTS], f32, tag="o30")
                    nc.scalar.activation(o30, ones30, AF.Copy, bias=float(bV), scale=v_t)
                    nc.sync.dma_start(out=out_d[rows, :], in_=o30)

            def body():
                for g in range(N_TILES // GROUP):
                    group_body(g)

            if loop_n > 1:
                # timing amplification: repeat the identical batch loop_n times
                with tc.For_i(0, loop_n, 1):
                    body()
            else:
                body()

    nc.compile()
    return nc


def _prep(inputs):
    import ml_dtypes

    xin = np.ascontiguousarray(inputs["inputs"], dtype=np.float32)
    xact = np.ascontiguousarray(inputs["actions"], dtype=np.float32)
    w1 = np.asarray(inputs["w1"], np.float32)
    b1 = np.asarray(inputs["b1"], np.float32)
    w2 = np.asarray(inputs["w2"], np.float32)
    b2 = np.asarray(inputs["b2"], np.float32)
    wV = np.asarray(inputs["wV"], np.float32)
    bV = float(np.asarray(inputs["bV"], np.float32).reshape(-1)[0])

    # LN affine params are identity in this model; the kernel folds them away.
    for k, want in (("g1", 1.0), ("g2", 1.0), ("beta1", 0.0), ("beta2", 0.0)):
        if k in inputs:
            assert np.allclose(np.asarray(inputs[k]), want), f"{k} must be {want}"

    bf = ml_dtypes.bfloat16
    w1t = np.ascontiguousarray(
        np.concatenate([w1, b1[:, None]], axis=1).T
    ).astype(bf)  # (15, 512)
    w2t = np.ascontiguousarray((N_AGENTS * w2).T).astype(bf)  # (512, 512)
    b2r = np.ascontiguousarray(b2[None, :]).astype(bf)  # (1, 512)
    wvr = np.ascontiguousarray(N_AGENTS * wV.reshape(1, -1), dtype=np.float32)

    in_maps = []
    for c in range(N_CORES):
        rows = slice(c * B_CORE, (c + 1) * B_CORE)
        in_maps.append(
            {
                "xin": xin[rows],
                "xact": xact[rows],
                "w1t": w1t,
                "w2t": w2t,
                "b2r": b2r,
                "wvr": wvr,
            }
        )
    return in_maps, bV


def _run(inputs, trace=False):
    from concourse.bass_utils import run_bass_kernel_spmd

    in_maps, bV = _prep(inputs)
    if "nc" not in _cache:
        _cache["nc"] = _build(bV)
    res = run_bass_kernel_spmd(
        _cache["nc"], in_maps, core_ids=list(range(N_CORES)), trace=trace
    )
    out = np.concatenate([m["out"] for m in res.results], axis=0)
    return out, res


def kernel(**inputs) -> np.ndarray:
    out, _ = _run(inputs, trace=False)
    return out


# revision 13
# speedup vs baseline: 1.3973x; 1.3973x over previous
"""Trainium2 Bass kernel for the tied-weight Critic MLP.

Math (derived from the reference):
  x   = concat(inputs, actions)                  (B, 420), B = 8192
  s   = sum over 30 column-blocks of 14          (B, 14)
  y1  = s @ W1.T + b1                            (B, 512)
  h1  = relu(layernorm_512(y1))        [g1=1, beta1=0, LN over the 30x tile
                                        equals LN over one 512 block]
  y2  = h1 @ (30*W2).T + b2                      (B, 512)
  h2  = relu(layernorm_512(y2))
  V   = h2 @ (30*wV).T + bV                      (B, 1)
  out = tile(V, 30)                              (B, 30)

Sharding: pure data parallelism - batch 8192 split as 1024 rows on each of
8 NeuronCores; weights replicated.

Per-core layout (batch-major, 8 tiles of 128 rows, two groups of 4 tiles):
  All transposes ride the DMA engines in bf16 (PE transposes and fp32
  matmuls are 2-4x slower on the PE): the four s blocks of a group are
  packed 32 partitions apart with a ones column (bias fold) and flipped by
  a single DMA transpose; mm1 runs as 4 bf16 matmuls against a 4x
  replicated weight tile; h1 is emitted in bf16 by the fused
  scale/bias-ReLU activation and flipped by 4 DMA transposes per tile; mm2
  is a K=1 b2-broadcast matmul plus 4 accumulating bf16 matmuls; LayerNorm
  uses bn_stats/bn_aggr; V is a fp32 mul+reduce against broadcast wV.
  Emission is phase-ordered across each group so the engines pipeline.
"""

import numpy as np

N_CORES = 8
B_FULL = 8192
B_CORE = B_FULL // N_CORES  # 1024
P = 128
N_TILES = B_CORE // P  # 8
GROUP = 4  # tiles per phase group
N_AGENTS = 30
IN_F = 14
HID = 512
EPS = 1e-5

_cache = {}


def _build(bV: float, loop_n: int = 1):
    import concourse.bass as bass
    import concourse.tile as tile
    from concourse import bacc, mybir
    from concourse.bass import ts

    f32 = mybir.dt.float32
    bf16 = mybir.dt.bfloat16
    AF = mybir.ActivationFunctionType
    ALU = mybir.AluOpType

    nc = bacc.Bacc("TRN2")

    xin_d = nc.dram_tensor("xin", (B_CORE, 360), f32, kind="ExternalInput")
    xact_d = nc.dram_tensor("xact", (B_CORE, 60), f32, kind="ExternalInput")
    w1t_d = nc.dram_tensor("w1t", (IN_F + 1, HID), bf16, kind="ExternalInput")
    w2t_d = nc.dram_tensor("w2t", (HID, HID), bf16, kind="ExternalInput")
    b2r_d = nc.dram_tensor("b2r", (1, HID), bf16, kind="ExternalInput")
    wvr_d = nc.dram_tensor("wvr", (1, HID), f32, kind="ExternalInput")
    out_d = nc.dram_tensor("out", (B_CORE, N_AGENTS), f32, kind="ExternalOutput")

    def bcast(ap, p=P):
        return bass.AP(tensor=ap.tensor, offset=ap.offset, ap=[[0, p]] + ap.ap[1:])

    with tile.TileContext(nc) as tc:
        with (
            tc.tile_pool(name="singles", bufs=1) as singles,
            tc.tile_pool(name="xp", bufs=2 * GROUP) as xp,
            tc.tile_pool(name="sp", bufs=2) as sp,
            tc.tile_pool(name="hp", bufs=GROUP) as hp,
            tc.tile_pool(name="stat", bufs=2 * GROUP) as stat,
            tc.tile_pool(name="op", bufs=GROUP) as op,
            tc.tile_pool(name="ps_y", bufs=GROUP, space="PSUM") as ps_y,
        ):
            # ---- constants / replicated weights ----
            ones30 = singles.tile([P, N_AGENTS], f32)
            nc.vector.memset(ones30, 1.0)
            ones1 = singles.tile([1, P], bf16)
            nc.vector.memset(ones1, 1.0)
            eps_t = singles.tile([P, 1], f32)
            nc.vector.memset(eps_t, EPS)

            # w1t replicated at partitions 0/32/64/96 (matmul needs lhsT and
            # rhs on the same base partition; the 4 sT slices sit 32 apart)
            w1t = singles.tile([96 + IN_F + 1, HID], bf16)
            for a in range(GROUP):
                nc.sync.dma_start(
                    out=w1t[32 * a : 32 * a + IN_F + 1, :], in_=w1t_d[:, :]
                )
            w2sb = singles.tile([P, 4, HID], bf16)
            nc.sync.dma_start(
                out=w2sb, in_=w2t_d[:, :].rearrange("(c p) n -> p c n", p=P)
            )
            b2r = singles.tile([1, HID], bf16)
            nc.sync.dma_start(out=b2r, in_=b2r_d[:, :])
            wv_bc = singles.tile([P, HID], f32)
            nc.gpsimd.dma_start(out=wv_bc, in_=bcast(wvr_d[:, :]))

            def layer_norm_relu(y_in, h_out):
                st6 = stat.tile([P, 6], f32, tag="st6")
                nc.vector.bn_stats(st6, y_in)
                mv = stat.tile([P, 2], f32, tag="mv")
                nc.vector.bn_aggr(mv, st6)
                rstd = stat.tile([P, 1], f32, tag="rstd")
                nc.scalar.activation(rstd, mv[:, 1:2], AF.Sqrt, bias=eps_t, scale=1.0)
                nc.vector.reciprocal(rstd, rstd)
                # nm = (mean * rstd) * -1
                nm = stat.tile([P, 1], f32, tag="nm")
                nc.vector.tensor_scalar(
                    out=nm,
                    in0=mv[:, 0:1],
                    scalar1=rstd,
                    scalar2=-1.0,
                    op0=ALU.mult,
                    op1=ALU.mult,
                )
                # h = relu(y * rstd + nm) = relu((y - mean) * rstd)
                nc.scalar.activation(h_out, y_in, AF.Relu, bias=nm, scale=rstd)

            def group_body(g):
                t0 = g * GROUP
                # ---- phase A: load x, form s, pack + single DMA transpose ----
                x_ts = []
                for a in range(GROUP):
                    rows = slice((t0 + a) * P, (t0 + a + 1) * P)
                    x_t = xp.tile([P, 420], f32, tag="x")
                    nc.scalar.dma_start(out=x_t[:, 0:360], in_=xin_d[rows, :])
                    nc.scalar.dma_start(out=x_t[:, 360:420], in_=xact_d[rows, :])
                    x_ts.append(x_t)
                s_f = sp.tile([P, GROUP, IN_F], f32, tag="sf")
                for a in range(GROUP):
                    nc.vector.reduce_sum(
                        s_f[:, a, :],
                        x_ts[a][:, :].rearrange("p (a f) -> p f a", f=IN_F),
                        axis=mybir.AxisListType.X,
                    )
                # s4b columns: tile a at 32a..32a+13, ones column at 32a+14
                # (becomes the bias row of each lhsT slice after transpose)
                s4b = sp.tile([P, P], bf16, tag="s4b")
                nc.vector.memset(s4b, 1.0)
                s4b_v = s4b[:, :].rearrange("p (a q) -> p a q", q=32)
                nc.scalar.copy(out=s4b_v[:, :, 0:IN_F], in_=s_f)
                st4 = sp.tile([P, P], bf16, tag="st4")
                nc.sync.dma_start_transpose(st4, s4b)

                # ---- phase B: mm1 ----
                y1s = []
                for a in range(GROUP):
                    y1 = ps_y.tile([P, HID], f32, tag="y1")
                    nc.tensor.matmul(
                        y1,
                        st4[32 * a : 32 * a + IN_F + 1, :],
                        w1t[32 * a : 32 * a + IN_F + 1, :],
                        start=True,
                        stop=True,
                        tile_position=(32 * a, 0),
                    )
                    y1s.append(y1)

                # ---- phase C: LN1 + ReLU -> bf16 ----
                h1bs = []
                for a in range(GROUP):
                    h1b = hp.tile([P, HID], bf16, tag="h1b")
                    layer_norm_relu(y1s[a], h1b)
                    h1bs.append(h1b)

                # ---- phase D: h1.T via DMA transposes ----
                h1ts = []
                for a in range(GROUP):
                    h1t = hp.tile([P, 4, P], bf16, tag="h1t")
                    for j in range(4):
                        nc.sync.dma_start_transpose(h1t[:, j, :], h1bs[a][:, ts(j, P)])
                    h1ts.append(h1t)

                # ---- phase E: mm2 = b2 (K=1) + 4 accumulating matmuls ----
                y2s = []
                for a in range(GROUP):
                    y2 = ps_y.tile([P, HID], f32, tag="y2")
                    nc.tensor.matmul(y2, ones1, b2r, start=True, stop=False)
                    for j in range(4):
                        nc.tensor.matmul(
                            y2,
                            h1ts[a][:, j, :],
                            w2sb[:, j, :],
                            start=False,
                            stop=(j == 3),
                        )
                    y2s.append(y2)

                # ---- phase F: LN2 + ReLU ----
                h2s = []
                for a in range(GROUP):
                    h2 = hp.tile([P, HID], f32, tag="h2")
                    layer_norm_relu(y2s[a], h2)
                    h2s.append(h2)

                # ---- phase G: V, broadcast to 30 cols, store ----
                for a in range(GROUP):
                    rows = slice((t0 + a) * P, (t0 + a + 1) * P)
                    tmp = hp.tile([P, HID], f32, tag="tmp")
                    v_t = stat.tile([P, 1], f32, tag="v")
                    nc.vector.tensor_mul(tmp, h2s[a], wv_bc)
                    nc.vector.reduce_sum(v_t, tmp, axis=mybir.AxisListType.X)
                    o30 = op.tile([P, N_AGENTS], f32, tag="o30")
                    nc.scalar.activation(o30, ones30, AF.Copy, bias=float(bV), scale=v_t)
                    nc.scalar.dma_start(out=out_d[rows, :], in_=o30)

            def body():
                for g in range(N_TILES // GROUP):
                    group_body(g)

            if loop_n > 1:
                # timing amplification: repeat the identical batch loop_n times
                with tc.For_i(0, loop_n, 1):
                    body()
            else:
                body()

    nc.compile()
    return nc


def _prep(inputs):
    import ml_dtypes

    xin = np.ascontiguousarray(inputs["inputs"], dtype=np.float32)
    xact = np.ascontiguousarray(inputs["actions"], dtype=np.float32)
    w1 = np.asarray(inputs["w1"], np.float32)
    b1 = np.asarray(inputs["b1"], np.float32)
    w2 = np.asarray(inputs["w2"], np.float32)
    b2 = np.asarray(inputs["b2"], np.float32)
    wV = np.asarray(inputs["wV"], np.float32)
    bV = float(np.asarray(inputs["bV"], np.float32).reshape(-1)[0])

    # LN affine params are identity in this model; the kernel folds them away.
    for k, want in (("g1", 1.0), ("g2", 1.0), ("beta1", 0.0), ("beta2", 0.0)):
        if k in inputs:
            assert np.allclose(np.asarray(inputs[k]), want), f"{k} must be {want}"

    bf = ml_dtypes.bfloat16
    w1t = np.ascontiguousarray(
        np.concatenate([w1, b1[:, None]], axis=1).T
    ).astype(bf)  # (15, 512)
    w2t = np.ascontiguousarray((N_AGENTS * w2).T).astype(bf)  # (512, 512)
    b2r = np.ascontiguousarray(b2[None, :]).astype(bf)  # (1, 512)
    wvr = np.ascontiguousarray(N_AGENTS * wV.reshape(1, -1), dtype=np.float32)

    in_maps = []
    for c in range(N_CORES):
        rows = slice(c * B_CORE, (c + 1) * B_CORE)
        in_maps.append(
            {
                "xin": xin[rows],
                "xact": xact[rows],
                "w1t": w1t,
                "w2t": w2t,
                "b2r": b2r,
                "wvr": wvr,
            }
        )
    return in_maps, bV


def _run(inputs, trace=False):
    from concourse.bass_utils import run_bass_kernel_spmd

    in_maps, bV = _prep(inputs)
    if "nc" not in _cache:
        _cache["nc"] = _build(bV)
    res = run_bass_kernel_spmd(
        _cache["nc"], in_maps, core_ids=list(range(N_CORES)), trace=trace
    )
    out = np.concatenate([m["out"] for m in res.results], axis=0)
    return out, res


def kernel(**inputs) -> np.ndarray:
    out, _ = _run(inputs, trace=False)
    return out


# revision 14
# speedup vs baseline: 1.6666x; 1.1927x over previous
"""Trainium2 Bass kernel for the tied-weight Critic MLP.

Math (derived from the reference):
  x   = concat(inputs, actions)                  (B, 420), B = 8192
  s   = sum over 30 column-blocks of 14          (B, 14)
  y1  = s @ W1.T + b1                            (B, 512)
  h1  = relu(layernorm_512(y1))        [g1=1, beta1=0, LN over the 30x tile
                                        equals LN over one 512 block]
  y2  = h1 @ (30*W2).T + b2                      (B, 512)
  h2  = relu(layernorm_512(y2))
  V   = h2 @ (30*wV).T + bV                      (B, 1)
  out = tile(V, 30)                              (B, 30)

Sharding: pure data parallelism - batch 8192 split as 1024 rows on each of
8 NeuronCores; weights replicated.

Per-core layout (batch-major, 8 tiles of 128 rows, two groups of 4 tiles):
  All transposes ride the DMA engines in bf16 (PE transposes and fp32
  matmuls are 2-4x slower on the PE): the four s blocks of a group are
  packed 32 partitions apart with a ones column (bias fold) and flipped by
  a single DMA transpose; mm1 runs as 4 bf16 matmuls against a 4x
  replicated weight tile; h1 is emitted in bf16 by the fused
  scale/bias-ReLU activation and flipped by 4 DMA transposes per tile; mm2
  is a K=1 b2-broadcast matmul plus 4 accumulating bf16 matmuls; LayerNorm
  uses bn_stats/bn_aggr; V is a fp32 mul+reduce against broadcast wV.
  Emission is phase-ordered across each group so the engines pipeline.
"""

import numpy as np

N_CORES = 8
B_FULL = 8192
B_CORE = B_FULL // N_CORES  # 1024
P = 128
N_TILES = B_CORE // P  # 8
GROUP = 4  # tiles per phase group
N_AGENTS = 30
IN_F = 14
HID = 512
EPS = 1e-5

_cache = {}


def _build(bV: float, loop_n: int = 1):
    import concourse.bass as bass
    import concourse.tile as tile
    from concourse import bacc, mybir
    from concourse.bass import ts

    f32 = mybir.dt.float32
    bf16 = mybir.dt.bfloat16
    AF = mybir.ActivationFunctionType
    ALU = mybir.AluOpType

    nc = bacc.Bacc("TRN2")

    xin_d = nc.dram_tensor("xin", (B_CORE, 360), f32, kind="ExternalInput")
    xact_d = nc.dram_tensor("xact", (B_CORE, 60), f32, kind="ExternalInput")
    w1t_d = nc.dram_tensor("w1t", (IN_F + 1, HID), bf16, kind="ExternalInput")
    w2t_d = nc.dram_tensor("w2t", (HID, HID), bf16, kind="ExternalInput")
    b2r_d = nc.dram_tensor("b2r", (1, HID), bf16, kind="ExternalInput")
    wvr_d = nc.dram_tensor("wvr", (1, HID), f32, kind="ExternalInput")
    out_d = nc.dram_tensor("out", (B_CORE, N_AGENTS), f32, kind="ExternalOutput")

    def bcast(ap, p=P):
        return bass.AP(tensor=ap.tensor, offset=ap.offset, ap=[[0, p]] + ap.ap[1:])

    with tile.TileContext(nc) as tc:
        with (
            tc.tile_pool(name="singles", bufs=1) as singles,
            tc.tile_pool(name="xp", bufs=2 * GROUP) as xp,
            tc.tile_pool(name="sp", bufs=2) as sp,
            tc.tile_pool(name="hp", bufs=GROUP) as hp,
            tc.tile_pool(name="stat", bufs=2 * GROUP) as stat,
            tc.tile_pool(name="op", bufs=GROUP) as op,
            tc.tile_pool(name="ps_y", bufs=GROUP, space="PSUM") as ps_y,
        ):
            # ---- constants / replicated weights ----
            ones30 = singles.tile([P, N_AGENTS], f32)
            nc.vector.memset(ones30, 1.0)
            ones1 = singles.tile([1, P], bf16)
            nc.vector.memset(ones1, 1.0)
            eps_t = singles.tile([P, 1], f32)
            nc.vector.memset(eps_t, EPS)

            # w1t replicated at partitions 0/32/64/96 (matmul needs lhsT and
            # rhs on the same base partition; the 4 sT slices sit 32 apart)
            w1t = singles.tile([96 + IN_F + 1, HID], bf16)
            for a in range(GROUP):
                nc.sync.dma_start(
                    out=w1t[32 * a : 32 * a + IN_F + 1, :], in_=w1t_d[:, :]
                )
            w2sb = singles.tile([P, 4, HID], bf16)
            nc.sync.dma_start(
                out=w2sb, in_=w2t_d[:, :].rearrange("(c p) n -> p c n", p=P)
            )
            b2r = singles.tile([1, HID], bf16)
            nc.sync.dma_start(out=b2r, in_=b2r_d[:, :])
            wv_bc = singles.tile([P, HID], f32)
            nc.gpsimd.dma_start(out=wv_bc, in_=bcast(wvr_d[:, :]))

            def layer_norm_relu(y_in, h_out):
                st6 = stat.tile([P, 6], f32, tag="st6")
                nc.vector.bn_stats(st6, y_in)
                mv = stat.tile([P, 2], f32, tag="mv")
                nc.vector.bn_aggr(mv, st6)
                rstd = stat.tile([P, 1], f32, tag="rstd")
                nc.scalar.activation(rstd, mv[:, 1:2], AF.Sqrt, bias=eps_t, scale=1.0)
                nc.vector.reciprocal(rstd, rstd)
                # nm = (mean * rstd) * -1
                nm = stat.tile([P, 1], f32, tag="nm")
                nc.vector.tensor_scalar(
                    out=nm,
                    in0=mv[:, 0:1],
                    scalar1=rstd,
                    scalar2=-1.0,
                    op0=ALU.mult,
                    op1=ALU.mult,
                )
                # h = relu(y * rstd + nm) = relu((y - mean) * rstd)
                nc.scalar.activation(h_out, y_in, AF.Relu, bias=nm, scale=rstd)

            def group_body(g):
                t0 = g * GROUP
                # ---- phase A: load x, form s, pack + single DMA transpose ----
                x_ts = []
                for a in range(GROUP):
                    rows = slice((t0 + a) * P, (t0 + a + 1) * P)
                    x_t = xp.tile([P, 420], f32, tag="x")
                    nc.gpsimd.dma_start(out=x_t[:, 0:360], in_=xin_d[rows, :])
                    nc.gpsimd.dma_start(out=x_t[:, 360:420], in_=xact_d[rows, :])
                    x_ts.append(x_t)
                s_f = sp.tile([P, GROUP, IN_F], f32, tag="sf")
                for a in range(GROUP):
                    nc.vector.reduce_sum(
                        s_f[:, a, :],
                        x_ts[a][:, :].rearrange("p (a f) -> p f a", f=IN_F),
                        axis=mybir.AxisListType.X,
                    )
                # s4b columns: tile a at 32a..32a+13, ones column at 32a+14
                # (becomes the bias row of each lhsT slice after transpose)
                s4b = sp.tile([P, P], bf16, tag="s4b")
                nc.vector.memset(s4b, 1.0)
                s4b_v = s4b[:, :].rearrange("p (a q) -> p a q", q=32)
                nc.scalar.copy(out=s4b_v[:, :, 0:IN_F], in_=s_f)
                st4 = sp.tile([P, P], bf16, tag="st4")
                nc.sync.dma_start_transpose(st4, s4b)

                # ---- phase B: mm1 ----
                y1s = []
                for a in range(GROUP):
                    y1 = ps_y.tile([P, HID], f32, tag="y1")
                    nc.tensor.matmul(
                        y1,
                        st4[32 * a : 32 * a + IN_F + 1, :],
                        w1t[32 * a : 32 * a + IN_F + 1, :],
                        start=True,
                        stop=True,
                        tile_position=(32 * a, 0),
                    )
                    y1s.append(y1)

                # ---- phase C: LN1 + ReLU -> bf16 ----
                h1bs = []
                for a in range(GROUP):
                    h1b = hp.tile([P, HID], bf16, tag="h1b")
                    layer_norm_relu(y1s[a], h1b)
                    h1bs.append(h1b)

                # ---- phase D: h1.T via DMA transposes ----
                h1ts = []
                for a in range(GROUP):
                    h1t = hp.tile([P, 4, P], bf16, tag="h1t")
                    for j in range(4):
                        nc.sync.dma_start_transpose(h1t[:, j, :], h1bs[a][:, ts(j, P)])
                    h1ts.append(h1t)

                # ---- phase E: mm2 = b2 (K=1) + 4 accumulating matmuls ----
                y2s = []
                for a in range(GROUP):
                    y2 = ps_y.tile([P, HID], f32, tag="y2")
                    nc.tensor.matmul(y2, ones1, b2r, start=True, stop=False)
                    for j in range(4):
                        nc.tensor.matmul(
                            y2,
                            h1ts[a][:, j, :],
                            w2sb[:, j, :],
                            start=False,
                            stop=(j == 3),
                        )
                    y2s.append(y2)

                # ---- phase F: LN2 + ReLU ----
                h2s = []
                for a in range(GROUP):
                    h2 = hp.tile([P, HID], f32, tag="h2")
                    layer_norm_relu(y2s[a], h2)
                    h2s.append(h2)

                # ---- phase G: V, broadcast to 30 cols, store ----
                for a in range(GROUP):
                    rows = slice((t0 + a) * P, (t0 + a + 1) * P)
                    tmp = hp.tile([P, HID], f32, tag="tmp")
                    v_t = stat.tile([P, 1], f32, tag="v")
                    nc.vector.tensor_mul(tmp, h2s[a], wv_bc)
                    nc.vector.reduce_sum(v_t, tmp, axis=mybir.AxisListType.X)
                    o30 = op.tile([P, N_AGENTS], f32, tag="o30")
                    nc.scalar.activation(o30, ones30, AF.Copy, bias=float(bV), scale=v_t)
                    nc.scalar.dma_start(out=out_d[rows, :], in_=o30)

            def body():
                for g in range(N_TILES // GROUP):
                    group_body(g)

            if loop_n > 1:
                # timing amplification: repeat the identical batch loop_n times
                with tc.For_i(0, loop_n, 1):
                    body()
            else:
                body()

    nc.compile()
    return nc


def _prep(inputs):
    import ml_dtypes

    xin = np.ascontiguousarray(inputs["inputs"], dtype=np.float32)
    xact = np.ascontiguousarray(inputs["actions"], dtype=np.float32)
    w1 = np.asarray(inputs["w1"], np.float32)
    b1 = np.asarray(inputs["b1"], np.float32)
    w2 = np.asarray(inputs["w2"], np.float32)
    b2 = np.asarray(inputs["b2"], np.float32)
    wV = np.asarray(inputs["wV"], np.float32)
    bV = float(np.asarray(inputs["bV"], np.float32).reshape(-1)[0])

    # LN affine params are identity in this model; the kernel folds them away.
    for k, want in (("g1", 1.0), ("g2", 1.0), ("beta1", 0.0), ("beta2", 0.0)):
        if k in inputs:
            assert np.allclose(np.asarray(inputs[k]), want), f"{k} must be {want}"

    bf = ml_dtypes.bfloat16
    w1t = np.ascontiguousarray(
        np.concatenate([w1, b1[:, None]], axis=1).T
    ).astype(bf)  # (15, 512)
    w2t = np.ascontiguousarray((N_AGENTS * w2).T).astype(bf)  # (512, 512)
    b2r = np.ascontiguousarray(b2[None, :]).astype(bf)  # (1, 512)
    wvr = np.ascontiguousarray(N_AGENTS * wV.reshape(1, -1), dtype=np.float32)

    in_maps = []
    for c in range(N_CORES):
        rows = slice(c * B_CORE, (c + 1) * B_CORE)
        in_maps.append(
            {
                "xin": xin[rows],
                "xact": xact[rows],
                "w1t": w1t,
                "w2t": w2t,
                "b2r": b2r,
                "wvr": wvr,
            }
        )
    return in_maps, bV


def _run(inputs, trace=False):
    from concourse.bass_utils import run_bass_kernel_spmd

    in_maps, bV = _prep(inputs)
    if "nc" not in _cache:
        _cache["nc"] = _build(bV)
    res = run_bass_kernel_spmd(
        _cache["nc"], in_maps, core_ids=list(range(N_CORES)), trace=trace
    )
    out = np.concatenate([m["out"] for m in res.results], axis=0)
    return out, res


def kernel(**inputs) -> np.ndarray:
    out, _ = _run(inputs, trace=False)
    return out
